# revision 1
# baseline (speedup 1.0000x reference)
"""Bass/Trainium2 kernel for LSTM(2-layer, H=256) + CRF Viterbi decode.

Contract: kernel(**inputs) takes the FULL unsharded inputs from
reference.setup_inputs() and returns (path_score, path) matching
reference.reference(**inputs).

Design (single-core program, replicated SPMD on 8 cores; output read from
core 0 — the computation is one inherently-serial sequence of T=8192 steps,
so data-parallelism across cores does not apply; per the sharding hint the
params are replicated and the serial recurrence runs on-core):

 - Embedding rows gathered per 64-step chunk via indirect DMA, transposed
   with the PE so the input projection is a [100,128]x[100,64] matmul.
 - LSTM recurrence: per step, gates = xp[t] + W_hh @ h as 16 self-loading
   fp32 matmuls [128,128]x[128,1] accumulating into a [128, 8] PSUM tile
   (gate order f,i,o,g; column m holds gate rows m*128..(m+1)*128).
 - The two LSTM layers are interleaved tick-by-tick (layer 1 lags layer 0
   by one 64-step chunk) so each layer's nonlinear tail hides under the
   other layer's weight streaming on the PE.
 - Viterbi forward pass runs entirely on the vector engine, lagging layer 1
   by one chunk: per step, a per-partition-scalar add (v + transitions), a
   32x32 DVE block transpose, max8 + max-index (argmax), and the emission
   add. Backpointers are streamed to DRAM per chunk.
 - The final argmax/backtrace over the [27, T] backpointer table is O(T)
   scalar pointer-chasing with zero FLOPs and runs on the host.
"""

import os
import numpy as np

NEG = -10000.0
NTAGS = 27
START, STOP = 25, 26
T = 8192
CH = 64
E = 100
H = 256
V = 200000

_PERM_BLOCKS = None


def _gate_perm():
    # torch gate order in the 4H weight rows: i, f, g, o (256 rows each).
    # Our layout order: f, i, o, g  (so sigmoid covers contiguous cols 0..5).
    global _PERM_BLOCKS
    if _PERM_BLOCKS is None:
        i0, f0, g0, o0 = 0, 256, 512, 768
        order = [f0, i0, o0, g0]
        _PERM_BLOCKS = np.concatenate([np.arange(b, b + 256) for b in order])
    return _PERM_BLOCKS


def _w_hh_image(w):
    # w: [1024, 256] -> SBUF image [128, 2*8*128]; tile (k, m) at columns
    # (k*8+m)*128 : +128 holds lhsT[kk, mm] = w[perm[m*128+mm], k*128+kk]
    perm = _gate_perm()
    wp = w[perm].reshape(8, 128, 256)  # [m, mm, kin]
    img = np.zeros((128, 2048), np.float32)
    for k in range(2):
        for m in range(8):
            img[:, (k * 8 + m) * 128 : (k * 8 + m + 1) * 128] = wp[
                m, :, k * 128 : (k + 1) * 128
            ].T
    return img


def _w_ih0_image(w):
    # w: [1024, 100] -> [100, 1024]; tile m at cols m*128 : +128
    perm = _gate_perm()
    wp = w[perm]  # [1024, 100]
    return np.ascontiguousarray(wp.reshape(8, 128, E).transpose(2, 0, 1).reshape(E, 1024)).astype(np.float32)


def _bias_image(b):
    perm = _gate_perm()
    return np.ascontiguousarray(b[perm].reshape(8, 128).T).astype(np.float32)


def _prep_inputs(sent, emb, w_ih0, w_hh0, b_ih0, b_hh0, w_ih1, w_hh1, b_ih1,
                 b_hh1, w_lin, b_lin, transitions, t_total, ch):
    nch = t_total // ch
    tp = (nch + 2) * ch
    sent_pad = np.zeros((tp,), np.int32)
    s = np.asarray(sent).reshape(-1).astype(np.int32)
    sent_pad[: s.shape[0]] = s

    wlinT = np.zeros((128, 2 * 32), np.float32)
    wl = np.asarray(w_lin, np.float32).T  # [256, 27]
    for k in range(2):
        wlinT[:, k * 32 : k * 32 + NTAGS] = wl[k * 128 : (k + 1) * 128]

    blin = np.zeros((32, 1), np.float32)
    blin[:NTAGS, 0] = np.asarray(b_lin, np.float32)

    trans = np.full((32, 32), NEG, np.float32)
    trans[:NTAGS, :NTAGS] = np.asarray(transitions, np.float32)

    v0 = np.full((32, 1), NEG, np.float32)
    v0[START, 0] = 0.0
    v0[NTAGS:, 0] = NEG

    ident = np.eye(ch, dtype=np.float32)

    return {
        "sent_pad": sent_pad,
        "emb": np.ascontiguousarray(np.asarray(emb, np.float32)),
        "wih0T": _w_ih0_image(np.asarray(w_ih0, np.float32)),
        "whh0T": _w_hh_image(np.asarray(w_hh0, np.float32)),
        "wih1T": _w_hh_image(np.asarray(w_ih1, np.float32)),
        "whh1T": _w_hh_image(np.asarray(w_hh1, np.float32)),
        "b0": _bias_image(np.asarray(b_ih0, np.float32) + np.asarray(b_hh0, np.float32)),
        "b1": _bias_image(np.asarray(b_ih1, np.float32) + np.asarray(b_hh1, np.float32)),
        "wlinT": wlinT,
        "blin": blin,
        "trans": trans,
        "v0": v0,
        "ident": ident,
    }


def build_program(t_total=T, ch=CH, num_devices=8):
    import concourse.bass as bass
    import concourse.tile as tile
    from concourse import bacc, mybir

    f32 = mybir.dt.float32
    i32 = mybir.dt.int32
    u16 = mybir.dt.uint16
    Alu = mybir.AluOpType
    Act = mybir.ActivationFunctionType
    ds = bass.ds

    nch = t_total // ch
    tp = (nch + 2) * ch

    nc = bacc.Bacc("TRN2", target_bir_lowering=False, debug=False,
                   enable_asserts=False, num_devices=num_devices)

    # ---- DRAM I/O ----
    d_sent = nc.dram_tensor("sent_pad", [tp], i32, kind="ExternalInput")
    d_emb = nc.dram_tensor("emb", [V, E], f32, kind="ExternalInput")
    d_wih0 = nc.dram_tensor("wih0T", [E, 1024], f32, kind="ExternalInput")
    d_whh0 = nc.dram_tensor("whh0T", [128, 2048], f32, kind="ExternalInput")
    d_wih1 = nc.dram_tensor("wih1T", [128, 2048], f32, kind="ExternalInput")
    d_whh1 = nc.dram_tensor("whh1T", [128, 2048], f32, kind="ExternalInput")
    d_b0 = nc.dram_tensor("b0", [128, 8], f32, kind="ExternalInput")
    d_b1 = nc.dram_tensor("b1", [128, 8], f32, kind="ExternalInput")
    d_wlin = nc.dram_tensor("wlinT", [128, 64], f32, kind="ExternalInput")
    d_blin = nc.dram_tensor("blin", [32, 1], f32, kind="ExternalInput")
    d_trans = nc.dram_tensor("trans", [32, 32], f32, kind="ExternalInput")
    d_v0 = nc.dram_tensor("v0", [32, 1], f32, kind="ExternalInput")
    d_ident = nc.dram_tensor("ident", [ch, ch], f32, kind="ExternalInput")

    d_bp = nc.dram_tensor("bp_out", [32, t_total], u16, kind="ExternalOutput")
    d_vT = nc.dram_tensor("vT_out", [32, 1], f32, kind="ExternalOutput")

    with tile.TileContext(nc) as tc:
        with (
            tc.tile_pool(name="consts", bufs=1) as consts,
            tc.tile_pool(name="state", bufs=1) as state,
            tc.tile_pool(name="xp", bufs=1) as xppool,
            tc.tile_pool(name="tmp", bufs=3) as tmp,
            tc.tile_pool(name="idx", bufs=2) as idxp,
            tc.tile_pool(name="pg0", bufs=2, space="PSUM") as pg0,
            tc.tile_pool(name="pg1", bufs=2, space="PSUM") as pg1,
            tc.tile_pool(name="psB", bufs=3, space="PSUM") as psB,
        ):
            # ---- load constants into SBUF ----
            whh0 = consts.tile([128, 2048], f32, tag="whh0")
            nc.sync.dma_start(whh0[:], d_whh0.ap()[:])
            whh1 = consts.tile([128, 2048], f32, tag="whh1")
            nc.sync.dma_start(whh1[:], d_whh1.ap()[:])
            wih1 = consts.tile([128, 2048], f32, tag="wih1")
            nc.sync.dma_start(wih1[:], d_wih1.ap()[:])
            wih0 = consts.tile([E, 1024], f32, tag="wih0")
            nc.sync.dma_start(wih0[:], d_wih0.ap()[:])
            wlin = consts.tile([128, 64], f32, tag="wlin")
            nc.sync.dma_start(wlin[:], d_wlin.ap()[:])
            b0 = consts.tile([128, 8], f32, tag="b0")
            nc.sync.dma_start(b0[:], d_b0.ap()[:])
            b1 = consts.tile([128, 8], f32, tag="b1")
            nc.sync.dma_start(b1[:], d_b1.ap()[:])
            blin = consts.tile([32, 1], f32, tag="blin")
            nc.sync.dma_start(blin[:], d_blin.ap()[:])
            trans = consts.tile([32, 32], f32, tag="trans")
            nc.sync.dma_start(trans[:], d_trans.ap()[:])
            ident = consts.tile([ch, ch], f32, tag="ident")
            nc.sync.dma_start(ident[:], d_ident.ap()[:])

            # ---- persistent state tiles ----
            v = state.tile([32, 1], f32, tag="v")
            nc.sync.dma_start(v[:], d_v0.ap()[:])
            c0 = state.tile([128, 2], f32, tag="c0")
            nc.vector.memset(c0[:], 0.0)
            c1 = state.tile([128, 2], f32, tag="c1")
            nc.vector.memset(c1[:], 0.0)
            sc_work = state.tile([32, 32], f32, tag="sc_work")
            nc.vector.memset(sc_work[:], NEG)
            scT = state.tile([32, 32], f32, tag="scT")
            nc.vector.memset(scT[:], NEG)
            vmax = state.tile([32, 8], f32, tag="vmax")
            nc.vector.memset(vmax[:], 0.0)
            vidx = state.tile([32, 8], u16, tag="vidx")
            nc.vector.memset(vidx[:], 0)
            bp_chunk = state.tile([32, ch], u16, tag="bp_chunk")
            nc.vector.memset(bp_chunk[:], 0)

            h0A = state.tile([128, 2 * ch], f32, tag="h0A")
            nc.vector.memset(h0A[:], 0.0)
            h0B = state.tile([128, 2 * ch], f32, tag="h0B")
            nc.vector.memset(h0B[:], 0.0)
            h1A = state.tile([128, 2 * ch], f32, tag="h1A")
            nc.vector.memset(h1A[:], 0.0)
            h1B = state.tile([128, 2 * ch], f32, tag="h1B")
            nc.vector.memset(h1B[:], 0.0)

            xp0A = xppool.tile([128, 8 * ch], f32, tag="xp0A")
            xp0B = xppool.tile([128, 8 * ch], f32, tag="xp0B")
            xp1A = xppool.tile([128, 8 * ch], f32, tag="xp1A")
            xp1B = xppool.tile([128, 8 * ch], f32, tag="xp1B")
            scA = state.tile([32, ch], f32, tag="scA")
            nc.vector.memset(scA[:], 0.0)
            scB = state.tile([32, ch], f32, tag="scB")
            nc.vector.memset(scB[:], 0.0)

            # ---------------- emitters ----------------

            def prep_xp0(sent_off_ap, xp_dst):
                """gather CH embedding rows + input projection into xp_dst."""
                sb_idx = idxp.tile([ch, 1], i32, tag="sb_idx")
                nc.sync.dma_start(
                    sb_idx[:, 0:1],
                    sent_off_ap.rearrange("(p one) -> p one", one=1),
                )
                x_rows = idxp.tile([ch, E], f32, tag="x_rows")
                nc.gpsimd.indirect_dma_start(
                    out=x_rows[:, :],
                    out_offset=None,
                    in_=d_emb.ap()[:, :],
                    in_offset=bass.IndirectOffsetOnAxis(ap=sb_idx[:, 0:1], axis=0),
                )
                ps_t = psB.tile([128, ch], f32, tag="psB")
                nc.tensor.transpose(ps_t[0:E, :], x_rows[:, :], ident[:, :])
                x_T = idxp.tile([E, ch], f32, tag="x_T")
                nc.vector.tensor_copy(x_T[:, :], ps_t[0:E, :])
                xv = xp_dst[:].rearrange("p (t g) -> p t g", g=8)
                for m in range(8):
                    ps = psB.tile([128, ch], f32, tag="psB")
                    nc.tensor.matmul(ps[:, :], wih0[:, m * 128 : (m + 1) * 128],
                                     x_T[:, :], start=True, stop=True)
                    nc.vector.tensor_scalar_add(xv[:, :, m], ps[:, :], b0[:, m : m + 1])

            def prep_xp1(h_src, xp_dst):
                """xp1 for one chunk from h0T chunk tile [128, 2*ch]."""
                hv = h_src[:].rearrange("p (t k) -> p k t", k=2)
                xv = xp_dst[:].rearrange("p (t g) -> p t g", g=8)
                for m in range(8):
                    ps = psB.tile([128, ch], f32, tag="psB")
                    nc.tensor.matmul(ps[:, :], wih1[:, m * 128 : (m + 1) * 128],
                                     hv[:, 0, :], start=True, stop=False)
                    nc.tensor.matmul(ps[:, :], wih1[:, (8 + m) * 128 : (9 + m) * 128],
                                     hv[:, 1, :], start=False, stop=True)
                    nc.vector.tensor_scalar_add(xv[:, :, m], ps[:, :], b1[:, m : m + 1])

            def emissions(h_src, sc_dst):
                hv = h_src[:].rearrange("p (t k) -> p k t", k=2)
                ps = psB.tile([128, ch], f32, tag="psB")
                nc.tensor.matmul(ps[0:NTAGS, :], wlin[:, 0:NTAGS], hv[:, 0, :],
                                 start=True, stop=False)
                nc.tensor.matmul(ps[0:NTAGS, :], wlin[:, 32 : 32 + NTAGS], hv[:, 1, :],
                                 start=False, stop=True)
                nc.vector.tensor_scalar_add(sc_dst[0:NTAGS, :], ps[0:NTAGS, :],
                                            blin[0:NTAGS, 0:1])

            def matvec(pg, w_sb, h_prev_lo, h_prev_hi):
                g = pg.tile([128, 8], f32, tag="g")
                for m in range(8):
                    nc.tensor.matmul(g[:, m : m + 1], w_sb[:, (m) * 128 : (m + 1) * 128],
                                     h_prev_lo, start=True, stop=False)
                    nc.tensor.matmul(g[:, m : m + 1], w_sb[:, (8 + m) * 128 : (9 + m) * 128],
                                     h_prev_hi, start=False, stop=True)
                return g

            def tail(g, xp_slice, c_tile, h_out):
                gs = tmp.tile([128, 8], f32, tag="gs")
                nc.vector.tensor_tensor(out=gs[:, :], in0=g[:, :], in1=xp_slice,
                                        op=Alu.add)
                ss = tmp.tile([128, 8], f32, tag="ss")
                nc.scalar.activation(ss[:, 0:6], gs[:, 0:6], Act.Sigmoid)
                nc.scalar.activation(ss[:, 6:8], gs[:, 6:8], Act.Tanh)
                t1 = tmp.tile([128, 2], f32, tag="t1")
                nc.vector.tensor_tensor(out=t1[:, :], in0=ss[:, 2:4], in1=ss[:, 6:8],
                                        op=Alu.mult)
                t2 = tmp.tile([128, 2], f32, tag="t2")
                nc.vector.tensor_tensor(out=t2[:, :], in0=ss[:, 0:2], in1=c_tile[:, :],
                                        op=Alu.mult)
                nc.vector.tensor_tensor(out=c_tile[:, :], in0=t1[:, :], in1=t2[:, :],
                                        op=Alu.add)
                th = tmp.tile([128, 2], f32, tag="th")
                nc.scalar.activation(th[:, :], c_tile[:, :], Act.Tanh)
                nc.vector.tensor_tensor(out=h_out, in0=ss[:, 4:6], in1=th[:, :],
                                        op=Alu.mult)

            def lstm_tick(layer, j, h_A, h_B, xp_buf, c_tile, w_sb):
                if j == 0:
                    lo = h_B[:, (ch - 1) * 2 : (ch - 1) * 2 + 1]
                    hi = h_B[:, (ch - 1) * 2 + 1 : (ch - 1) * 2 + 2]
                else:
                    lo = h_A[:, (j - 1) * 2 : (j - 1) * 2 + 1]
                    hi = h_A[:, (j - 1) * 2 + 1 : (j - 1) * 2 + 2]
                pg = pg0 if layer == 0 else pg1
                g = matvec(pg, w_sb, lo, hi)
                tail(g, xp_buf[:, j * 8 : (j + 1) * 8], c_tile,
                     h_A[:, j * 2 : (j + 1) * 2])

            def vit_tick(j):
                nc.vector.tensor_scalar_add(sc_work[0:NTAGS, 0:NTAGS],
                                            trans[0:NTAGS, 0:NTAGS], v[0:NTAGS, 0:1])
                nc.vector.transpose(scT[:, :], sc_work[:, :])
                nc.vector.max(vmax[0:NTAGS, 0:8], scT[0:NTAGS, 0:NTAGS])
                nc.vector.max_index(vidx[0:NTAGS, 0:8], vmax[0:NTAGS, 0:8],
                                    scT[0:NTAGS, 0:NTAGS])
                nc.vector.tensor_copy(bp_chunk[0:NTAGS, j : j + 1], vidx[0:NTAGS, 0:1])
                nc.vector.tensor_tensor(out=v[0:NTAGS, 0:1], in0=vmax[0:NTAGS, 0:1],
                                        in1=scA[0:NTAGS, j : j + 1], op=Alu.add)

            def copy_carries():
                nc.vector.tensor_copy(h0B[:, (ch - 1) * 2 : ch * 2],
                                      h0A[:, (ch - 1) * 2 : ch * 2])
                nc.vector.tensor_copy(h1B[:, (ch - 1) * 2 : ch * 2],
                                      h1A[:, (ch - 1) * 2 : ch * 2])
                nc.vector.tensor_copy(xp0A[:], xp0B[:])
                nc.vector.tensor_copy(xp1A[:], xp1B[:])
                nc.vector.tensor_copy(scA[:], scB[:])

            # ---------------- prologue ----------------
            sent_ap = d_sent.ap()
            prep_xp0(sent_ap[0 * ch : 1 * ch], xp0A)
            prep_xp0(sent_ap[1 * ch : 2 * ch], xp0B)
            # layer-0 chunk 0
            for j in range(ch):
                lstm_tick(0, j, h0A, h0B, xp0A, c0, whh0)
            prep_xp1(h0A, xp1A)
            nc.vector.tensor_copy(h0B[:, (ch - 1) * 2 : ch * 2],
                                  h0A[:, (ch - 1) * 2 : ch * 2])
            nc.vector.tensor_copy(xp0A[:], xp0B[:])
            prep_xp0(sent_ap[2 * ch : 3 * ch], xp0B)
            # layer-0 chunk 1 / layer-1 chunk 0
            for j in range(ch):
                lstm_tick(0, j, h0A, h0B, xp0A, c0, whh0)
                lstm_tick(1, j, h1A, h1B, xp1A, c1, whh1)
            prep_xp1(h0A, xp1B)
            emissions(h1A, scB)
            copy_carries()

            # ---------------- main loop ----------------
            # body(i) for i in [2, nch]: off = (i-2)*ch
            n_iters = nch - 1  # i = 2 .. nch
            sent_shift = sent_ap[3 * ch : (nch + 2) * ch]
            with tc.For_i(0, n_iters * ch, ch) as off:
                prep_xp0(sent_shift[ds(off, ch)], xp0B)
                for j in range(ch):
                    lstm_tick(0, j, h0A, h0B, xp0A, c0, whh0)
                    lstm_tick(1, j, h1A, h1B, xp1A, c1, whh1)
                    vit_tick(j)
                nc.sync.dma_start(d_bp.ap()[:, ds(off, ch)], bp_chunk[:])
                emissions(h1A, scB)
                prep_xp1(h0A, xp1B)
                copy_carries()

            # ---------------- epilogue: viterbi chunk nch-1 ----------------
            for j in range(ch):
                vit_tick(j)
            nc.sync.dma_start(d_bp.ap()[:, (nch - 1) * ch : nch * ch], bp_chunk[:])
            nc.sync.dma_start(d_vT.ap()[:], v[:])

    nc.compile()
    return nc


_CACHE = {}


def _get_program(t_total, ch, num_devices):
    key = (t_total, ch, num_devices)
    if key not in _CACHE:
        _CACHE[key] = build_program(t_total, ch, num_devices)
    return _CACHE[key]


def kernel(**inputs):
    from concourse import bass_utils

    n_cores = 8
    sent = np.asarray(inputs["sent"]).reshape(-1)
    t_total = sent.shape[0]
    transitions = np.asarray(inputs["transitions"], np.float32)
    pre = _prep_inputs(
        sent, inputs["emb"], inputs["w_ih0"], inputs["w_hh0"], inputs["b_ih0"],
        inputs["b_hh0"], inputs["w_ih1"], inputs["w_hh1"], inputs["b_ih1"],
        inputs["b_hh1"], inputs["w_lin"], inputs["b_lin"], transitions,
        t_total, CH,
    )
    nc = _get_program(t_total, CH, n_cores)
    in_maps = [dict(pre) for _ in range(n_cores)]
    res = bass_utils.run_bass_kernel_spmd(nc, in_maps, core_ids=list(range(n_cores)))
    out = res.results[0]
    bp = out["bp_out"][:NTAGS].astype(np.int64)  # [27, T]
    vT = out["vT_out"][:NTAGS, 0].astype(np.float32)

    term = vT + transitions[:, STOP]
    best = int(np.argmax(term))
    path_score = np.float32(term[best])
    path = np.empty(t_total, np.int32)
    tag = best
    for t in range(t_total - 1, -1, -1):
        path[t] = tag
        tag = int(bp[tag, t])
    return path_score, path
